# revision 1
# baseline (speedup 1.0000x reference)
"""Trainium2 Bass kernel for nn_ContextQueryAttentionLayer.

Math: with B,N,M,D = 32,1024,256,128 the reference's gather index collapses:
  idx[i,j] = (i*M + j) % N = 256*(i%4) + j          (since M=256, N=1024)
so the similarity matrix S (b,n,m) has only 4 distinct rows per batch,
S[b,i,:] = t[b, i%4, :] with t (4,256):
  t[r,j] = q_j.w_q + c_{256r+j}.w_c + sum_d q_{j,d} w_m_d c_{256r+j,d}
Both softmaxes, c2q, sm (reduces to a 4x4 matrix per batch) and q2c then
collapse to rank-4-per-batch quantities, leaving a DMA-bound kernel:
  out[b,n] = [ctx_n, C2Q[n%4], ctx_n*C2Q[n%4], ctx_n*Q2C[n%4]]

Sharding: data-parallel over batch, 4 batches per core on 8 cores.
On-core layout: rows n=128k+p -> partition p (so n%4 == p%4). Query-only
prep (qwc, s_q) and the context column-sum tree are batched across all 4
resident batches; the per-batch t-columns pipeline POOL multiplies into DVE
reduces, one PE transpose moves t into an (8,128) softmax domain where all
scalings are per-partition, and the batch tail (products of context with
the broadcast C2Q/Q2C rows) is written by split DMA streams so no on-chip
assembly copies are needed.
"""

import numpy as np

B, N, M, D = 32, 1024, 256, 128
NCORES = 8
BPC = B // NCORES  # batches per core

_prog = None

# packed constant layout: name -> (partitions, col_start, col_len)
_CST_COLS = {
    "ident": (128, 0, 128),
    "wmb": (128, 128, 128),
    "wcb": (128, 256, 128),
    "wqb": (128, 384, 128),
    "b4": (4, 512, 128),
    "i16": (16, 640, 16),
    "pairsel": (16, 656, 8),
    "pairselT": (8, 664, 16),
    "hsel": (16, 680, 4),
    "rsel": (128, 700, 4),
}
_CST_W = 704


def _build_program():
    import concourse.bacc as bacc
    import concourse.mybir as mybir
    from concourse.tile import TileContext

    fp32 = mybir.dt.float32
    nc = bacc.Bacc("TRN2", target_bir_lowering=False, name="cqattn")

    ctx_d = nc.dram_tensor("ctx", [BPC, N, D], fp32, kind="ExternalInput")
    qry_d = nc.dram_tensor("qry", [BPC, M, D], fp32, kind="ExternalInput")
    cstp_d = nc.dram_tensor("cstp", [128, _CST_W], fp32, kind="ExternalInput")
    out_d = nc.dram_tensor("out", [BPC, N, 4 * D], fp32, kind="ExternalOutput")

    Exp = mybir.ActivationFunctionType.Exp
    Copy = mybir.ActivationFunctionType.Copy
    add = mybir.AluOpType.add
    X = mybir.AxisListType.X

    with TileContext(nc) as tc:
        with (
            tc.tile_pool(name="consts", bufs=1) as consts,
            tc.tile_pool(name="io", bufs=1) as io,
            tc.tile_pool(name="work", bufs=2) as work,
            tc.tile_pool(name="small", bufs=2) as small,
            tc.tile_pool(name="outp", bufs=2) as outp,
            tc.tile_pool(name="ps_tr", bufs=2, space="PSUM") as ps_tr,
            tc.tile_pool(name="ps_sm", bufs=1, space="PSUM") as ps_sm,
            tc.tile_pool(name="ps_mm", bufs=2, space="PSUM") as ps_mm,
            tc.tile_pool(name="ps_cs", bufs=1, space="PSUM") as ps_cs,
            tc.tile_pool(name="ps_rep", bufs=2, space="PSUM") as ps_rep,
        ):
            cstp = consts.tile([128, _CST_W], fp32, tag="cstp", name="cstp")
            nc.sync.dma_start(out=cstp, in_=cstp_d[...])
            cst = {
                n: cstp[:p, c0 : c0 + cl] for n, (p, c0, cl) in _CST_COLS.items()
            }

            # ---- all loads up front: rows n=128k+p -> partition p, block k
            ctx_mega = io.tile([128, BPC, 8, 128], fp32, tag="ctx", name="ctx_mega")
            qry_mega = io.tile([128, BPC, 2, 128], fp32, tag="qry", name="qry_mega")
            for b in range(BPC):
                nc.sync.dma_start(
                    out=ctx_mega[:, b],
                    in_=ctx_d[b].rearrange("(k p) d -> p k d", p=128),
                )
                nc.sync.dma_start(
                    out=qry_mega[:, b],
                    in_=qry_d[b].rearrange("(h p) d -> p h d", p=128),
                )
                # output stream a: raw context columns (no compute needed)
                nc.scalar.dma_start(
                    out=out_d[b][:, 0:128].rearrange("(k p) c -> p k c", p=128),
                    in_=ctx_mega[:, b],
                )

            # ---- batched query prep: qwcT = qry*w_m + w_c, sq = qry . w_q
            qwcT = work.tile([128, BPC, 2, 128], fp32, tag="qwcT")
            nc.vector.tensor_mul(
                qwcT,
                qry_mega,
                cst["wmb"]
                .rearrange("p (u v d) -> p u v d", u=1, v=1)
                .to_broadcast([128, BPC, 2, 128]),
            )
            nc.vector.tensor_add(
                qwcT,
                qwcT,
                cst["wcb"]
                .rearrange("p (u v d) -> p u v d", u=1, v=1)
                .to_broadcast([128, BPC, 2, 128]),
            )
            sq_tmp = work.tile([128, BPC, 2, 128], fp32, tag="sq_tmp")
            nc.vector.tensor_mul(
                sq_tmp,
                qry_mega,
                cst["wqb"]
                .rearrange("p (u v d) -> p u v d", u=1, v=1)
                .to_broadcast([128, BPC, 2, 128]),
            )
            sq_col = small.tile([128, BPC, 2], fp32, tag="sq_col")
            nc.vector.tensor_reduce(out=sq_col, in_=sq_tmp, axis=X, op=add)

            # ---- batched CS tree: csum[p,b,d] = sum_k ctx[b,128k+p,d]
            tmp4 = work.tile([128, BPC, 4, 128], fp32, tag="tmp4")
            nc.vector.tensor_add(
                tmp4, ctx_mega[:, :, 0:4, :], ctx_mega[:, :, 4:8, :]
            )
            tmp2 = work.tile([128, BPC, 2, 128], fp32, tag="tmp2")
            nc.gpsimd.tensor_add(tmp2, tmp4[:, :, 0:2, :], tmp4[:, :, 2:4, :])
            csum = work.tile([128, BPC, 128], fp32, tag="csum")
            nc.gpsimd.tensor_add(csum, tmp2[:, :, 0, :], tmp2[:, :, 1, :])
            cs_ps = ps_cs.tile([4, BPC, 128], fp32, tag="cs")
            nc.tensor.matmul(cs_ps, cst["rsel"], csum, start=True, stop=True)
            cs = small.tile([4, BPC, 128], fp32, tag="cs")
            nc.scalar.copy(out=cs, in_=cs_ps)

            for b in range(BPC):
                ctx_b = ctx_mega[:, b]
                qry_b = qry_mega[:, b]

                # ---- t columns: t_sb[p, 2r+h] = t[r, 128h+p]
                # POOL multiplies, DVE reduces (pipelined per h)
                t_sb = small.tile([128, 8], fp32, tag="t_sb")
                ctx_v = ctx_b.rearrange("p (r h) d -> p h r d", h=2)
                t_v = t_sb[:, :].rearrange("p (r h) -> p h r", h=2)
                for h in range(2):
                    g_tmp = work.tile([128, 4, 128], fp32, tag="g_tmp")
                    nc.gpsimd.tensor_mul(
                        g_tmp,
                        ctx_v[:, h],
                        qwcT[:, b, h, :]
                        .rearrange("p (u d) -> p u d", u=1)
                        .to_broadcast([128, 4, 128]),
                    )
                    nc.vector.tensor_reduce(
                        out=t_v[:, h], in_=g_tmp, axis=X, op=add
                    )
                nc.vector.tensor_add(
                    t_sb[:, :].rearrange("p (r h) -> p r h", h=2),
                    t_sb[:, :].rearrange("p (r h) -> p r h", h=2),
                    sq_col[:, b, :]
                    .rearrange("p (u h) -> p u h", u=1)
                    .to_broadcast([128, 4, 2]),
                )

                # ---- transpose to (8,128): row q = 2r+h, free p
                t8_ps = ps_tr.tile([8, 128], fp32, tag="tr")
                nc.tensor.transpose(t8_ps, t_sb, cst["ident"])

                # ---- softmaxes (no max-shift: |t| < ~8)
                e8 = small.tile([8, 128], fp32, tag="e8")
                rowsumc = small.tile([8, 1], fp32, tag="rowsumc")
                nc.scalar.activation(out=e8, in_=t8_ps, func=Exp, accum_out=rowsumc)
                # soft_c scale: rowsums per r via pairsel, reciprocal, broadcast
                pairs_ps = ps_sm.tile([4, 1], fp32, tag="sm")
                nc.tensor.matmul(
                    pairs_ps, cst["pairsel"][:8, :4], rowsumc, start=True, stop=True
                )
                rec4 = small.tile([4, 1], fp32, tag="rec4")
                nc.vector.reciprocal(out=rec4, in_=pairs_ps)
                rec8_ps = ps_sm.tile([8, 1], fp32, tag="sm")
                nc.tensor.matmul(
                    rec8_ps, cst["pairselT"][:4, :8], rec4, start=True, stop=True
                )
                rec8 = small.tile([8, 1], fp32, tag="rec8")
                nc.vector.tensor_copy(out=rec8, in_=rec8_ps)
                sc8 = small.tile([8, 128], fp32, tag="sc8")
                nc.scalar.activation(out=sc8, in_=e8, func=Copy, scale=rec8)
                # soft_q denominators: u2[h,p] = sum_r e8[2r+h,p]
                u2_ps = ps_sm.tile([2, 128], fp32, tag="sm")
                nc.tensor.matmul(
                    u2_ps, cst["hsel"][:8, :2], e8, start=True, stop=True
                )
                u2 = small.tile([2, 128], fp32, tag="u2")
                nc.scalar.copy(out=u2, in_=u2_ps)

                # ---- transposed-domain soft rows (128, 8): col q = 2r+h
                scT_ps = ps_tr.tile([128, 8], fp32, tag="tr")
                nc.tensor.transpose(scT_ps, sc8, cst["i16"][:8, :8])
                scT2 = small.tile([128, 8], fp32, tag="scT")
                nc.vector.tensor_copy(out=scT2, in_=scT_ps)
                scT = scT2[:, :].rearrange("p (r h) -> p r h", r=4)
                eT_ps = ps_tr.tile([128, 8], fp32, tag="tr")
                nc.tensor.transpose(eT_ps, e8, cst["i16"][:8, :8])
                u2T_ps = ps_tr.tile([128, 2], fp32, tag="tr")
                nc.tensor.transpose(u2T_ps, u2, cst["i16"][:2, :2])
                recu = small.tile([128, 2], fp32, tag="recu")
                nc.vector.reciprocal(out=recu, in_=u2T_ps)
                sqT2 = small.tile([128, 8], fp32, tag="sqT")
                nc.vector.tensor_mul(
                    sqT2[:, :].rearrange("p (r h) -> p r h", r=4),
                    eT_ps[:, :].rearrange("p (r h) -> p r h", r=4),
                    recu[:, :]
                    .rearrange("p (u h) -> p u h", u=1)
                    .to_broadcast([128, 4, 2]),
                )
                sqT = sqT2[:, :].rearrange("p (r h) -> p r h", r=4)

                # ---- SM4T[r',r] = sum_j sq[r',j] sc[r,j], scaled by 1/256
                sm4t_ps = ps_mm.tile([4, 4], fp32, tag="mm")
                for h in range(2):
                    nc.tensor.matmul(
                        sm4t_ps, sqT[:, :, h], scT[:, :, h],
                        start=(h == 0), stop=(h == 1),
                    )
                sm4t = small.tile([4, 4], fp32, tag="sm4t")
                nc.vector.tensor_scalar_mul(sm4t, sm4t_ps, 1.0 / 256.0)

                # ---- C2Q[r,d] = sum_j sc[r,j] qry[j,d]
                c2q_ps = ps_mm.tile([4, 128], fp32, tag="mm")
                for h in range(2):
                    nc.tensor.matmul(
                        c2q_ps, scT[:, :, h], qry_b[:, h, :],
                        start=(h == 0), stop=(h == 1),
                    )
                c2q = small.tile([4, 128], fp32, tag="c2q")
                nc.scalar.copy(out=c2q, in_=c2q_ps)

                # ---- Q2C[r,d] = sum_{r'} SM4[r,r'] CS[r',d]
                q2c_ps = ps_mm.tile([4, 128], fp32, tag="mm")
                nc.tensor.matmul(q2c_ps, sm4t, cs[:, b, :], start=True, stop=True)
                q2c = small.tile([4, 128], fp32, tag="q2c")
                nc.scalar.copy(out=q2c, in_=q2c_ps)

                # ---- broadcast rows r -> 128 partitions (p%4 pattern)
                repc_ps = ps_rep.tile([128, 128], fp32, tag="rep")
                nc.tensor.matmul(repc_ps, cst["b4"], c2q, start=True, stop=True)
                repc = small.tile([128, 128], fp32, tag="repc")
                nc.scalar.copy(out=repc, in_=repc_ps)
                repq_ps = ps_rep.tile([128, 128], fp32, tag="rep")
                nc.tensor.matmul(repq_ps, cst["b4"], q2c, start=True, stop=True)
                repq = small.tile([128, 128], fp32, tag="repq")
                nc.scalar.copy(out=repq, in_=repq_ps)

                # ---- output streams b (broadcast C2Q cols) and c/d (products)
                nc.scalar.dma_start(
                    out=out_d[b][:, 128:256].rearrange("(k p) c -> p k c", p=128),
                    in_=repc[:, :]
                    .rearrange("p (u d) -> p u d", u=1)
                    .to_broadcast([128, 8, 128]),
                )
                out_sb = outp.tile([128, 8, 2, 128], fp32, tag="out")
                nc.vector.tensor_mul(
                    out_sb[:, :, 0, :],
                    ctx_b,
                    repc[:, :]
                    .rearrange("p (u d) -> p u d", u=1)
                    .to_broadcast([128, 8, 128]),
                )
                nc.sync.dma_start(
                    out=out_d[b][:, 256:384].rearrange("(k p) c -> p k c", p=128),
                    in_=out_sb[:, :, 0, :],
                )
                eng = nc.vector if b == BPC - 1 else nc.gpsimd
                eng.tensor_mul(
                    out_sb[:, :, 1, :],
                    ctx_b,
                    repq[:, :]
                    .rearrange("p (u d) -> p u d", u=1)
                    .to_broadcast([128, 8, 128]),
                )
                nc.sync.dma_start(
                    out=out_d[b][:, 384:512].rearrange("(k p) c -> p k c", p=128),
                    in_=out_sb[:, :, 1, :],
                )
    nc.compile()
    return nc


def _get_program():
    global _prog
    if _prog is None:
        _prog = _build_program()
    return _prog


def _make_const_inputs(w):
    w = np.ascontiguousarray(w, dtype=np.float32)
    w_q, w_c, w_m = w[:D, 0], w[D : 2 * D, 0], w[2 * D :, 0]
    p = np.arange(128)
    q = np.arange(16)
    pairsel = (q[:, None] // 2 == np.arange(8)[None, :]).astype(np.float32)
    hsel = (
        2 * (q[:, None] // 8) + (q[:, None] % 2) == np.arange(4)[None, :]
    ).astype(np.float32)
    vals = {
        "ident": np.eye(128, dtype=np.float32),
        "i16": np.eye(16, dtype=np.float32),
        "wmb": np.broadcast_to(w_m[None, :], (128, 128)),
        "wcb": np.broadcast_to(w_c[None, :], (128, 128)),
        "wqb": np.broadcast_to(w_q[None, :], (128, 128)),
        "pairsel": pairsel,
        "pairselT": pairsel.T,
        "hsel": hsel,
        "rsel": (p[:, None] % 4 == np.arange(4)[None, :]).astype(np.float32),
        "b4": (np.arange(4)[:, None] == p[None, :] % 4).astype(np.float32),
    }
    packed = np.zeros((128, _CST_W), dtype=np.float32)
    for n, (parts, c0, cl) in _CST_COLS.items():
        packed[:parts, c0 : c0 + cl] = vals[n]
    return {"cstp": packed}


def _run(context, query, w, trace=False):
    from concourse.bass_utils import run_bass_kernel_spmd

    nc = _get_program()
    context = np.ascontiguousarray(context, dtype=np.float32)
    query = np.ascontiguousarray(query, dtype=np.float32)
    consts = _make_const_inputs(w)

    in_maps = []
    for c in range(NCORES):
        m = {
            "ctx": context[c * BPC : (c + 1) * BPC],
            "qry": query[c * BPC : (c + 1) * BPC],
        }
        m.update(consts)
        in_maps.append(m)

    res = run_bass_kernel_spmd(
        nc, in_maps, core_ids=list(range(NCORES)), trace=trace
    )
    out = np.concatenate([res.results[c]["out"] for c in range(NCORES)], axis=0)
    return out, res


def kernel(context, query, c_mask, q_mask, w):
    out, _ = _run(context, query, w, trace=False)
    return out



# revision 7
# speedup vs baseline: 1.0520x; 1.0520x over previous
"""Trainium2 Bass kernel for nn_ContextQueryAttentionLayer.

Math: with B,N,M,D = 32,1024,256,128 the reference's gather index collapses:
  idx[i,j] = (i*M + j) % N = 256*(i%4) + j          (since M=256, N=1024)
so the similarity matrix S (b,n,m) has only 4 distinct rows per batch,
S[b,i,:] = t[b, i%4, :] with t (4,256):
  t[r,j] = q_j.w_q + c_{256r+j}.w_c + sum_d q_{j,d} w_m_d c_{256r+j,d}
Both softmaxes, c2q, sm (reduces to a 4x4 matrix per batch) and q2c then
collapse to rank-4-per-batch quantities, leaving a DMA-bound kernel:
  out[b,n] = [ctx_n, C2Q[n%4], ctx_n*C2Q[n%4], ctx_n*Q2C[n%4]]

Sharding: data-parallel over batch, 4 batches per core on 8 cores.
On-core layout: rows n=128k+p -> partition p (so n%4 == p%4).

v3: output rows assembled in SBUF ([128,8,512]/batch, ctx DMA-loaded
straight into cols 0:128) -> ONE output DMA per batch with 2KB-contiguous
descriptors. qry loads issue before ctx so query prep overlaps the ctx
loads. Softmax runs in two 2-batch groups ([16,128] domain, block-diag
selection matmuls) so group-0 output DMAs launch while group 1 still
computes. Column sums via TensorE (rsel matmul) instead of an add tree.
Issue order tracks expected readiness (engines execute in order).
"""

import numpy as np

B, N, M, D = 32, 1024, 256, 128
NCORES = 8
BPC = B // NCORES  # batches per core

_prog = None

# packed constant layout: name -> (partitions, col_start, col_len)
_CST_COLS = {
    "ident": (128, 0, 128),
    "wmb": (128, 128, 128),
    "wcb": (128, 256, 128),
    "wqb": (128, 384, 128),
    "b4": (4, 512, 128),
    "i16": (16, 640, 16),
    "pairsel": (16, 656, 8),
    "pairselT": (8, 664, 16),
    "hsel": (16, 680, 4),
    "rsel": (128, 684, 4),
}
_CST_W = 688


def _build_program():
    import concourse.bacc as bacc
    import concourse.mybir as mybir
    from concourse.tile import TileContext

    fp32 = mybir.dt.float32
    nc = bacc.Bacc("TRN2", target_bir_lowering=False, name="cqattn")

    ctx_d = nc.dram_tensor("ctx", [BPC, N, D], fp32, kind="ExternalInput")
    qry_d = nc.dram_tensor("qry", [BPC, M, D], fp32, kind="ExternalInput")
    cstp_d = nc.dram_tensor("cstp", [128, _CST_W], fp32, kind="ExternalInput")
    out_d = nc.dram_tensor("out", [BPC, N, 4 * D], fp32, kind="ExternalOutput")

    Exp = mybir.ActivationFunctionType.Exp
    Copy = mybir.ActivationFunctionType.Copy
    add = mybir.AluOpType.add
    X = mybir.AxisListType.X

    with TileContext(nc) as tc:
        with (
            tc.tile_pool(name="consts", bufs=1) as consts,
            tc.tile_pool(name="io", bufs=1) as io,
            tc.tile_pool(name="work", bufs=2) as work,
            tc.tile_pool(name="small", bufs=2) as small,
            tc.tile_pool(name="outp", bufs=1) as outp,
            tc.tile_pool(name="ps_tr", bufs=2, space="PSUM") as ps_tr,
            tc.tile_pool(name="ps_sm", bufs=1, space="PSUM") as ps_sm,
            tc.tile_pool(name="ps_mm", bufs=2, space="PSUM") as ps_mm,
            tc.tile_pool(name="ps_cs", bufs=1, space="PSUM") as ps_cs,
            tc.tile_pool(name="ps_rep", bufs=2, space="PSUM") as ps_rep,
        ):
            cstp = consts.tile([128, _CST_W], fp32, tag="cstp", name="cstp")
            nc.sync.dma_start(out=cstp, in_=cstp_d[...])
            cst = {
                n: cstp[:p, c0 : c0 + cl] for n, (p, c0, cl) in _CST_COLS.items()
            }

            # ---- loads: qry first (gates query prep), then ctx per batch.
            # ctx lands directly in the output-assembly tiles (cols 0:128).
            qry_mega = io.tile([128, BPC, 2, 128], fp32, tag="qry", name="qry_mega")
            for b in range(BPC):
                nc.sync.dma_start(
                    out=qry_mega[:, b],
                    in_=qry_d[b].rearrange("(h p) d -> p h d", p=128),
                )
            out_sb = [
                outp.tile([128, 8, 512], fp32, tag=f"out{b}", name=f"out{b}")
                for b in range(BPC)
            ]
            for b in range(BPC):
                nc.sync.dma_start(
                    out=out_sb[b][:, :, 0:128],
                    in_=ctx_d[b].rearrange("(k p) d -> p k d", p=128),
                )

            # ---- batched query prep: qwcT = qry*w_m + w_c, sq = qry . w_q
            qwcT = work.tile([128, BPC, 2, 128], fp32, tag="qwcT")
            nc.vector.tensor_mul(
                qwcT,
                qry_mega,
                cst["wmb"]
                .rearrange("p (u v d) -> p u v d", u=1, v=1)
                .to_broadcast([128, BPC, 2, 128]),
            )
            nc.gpsimd.tensor_add(
                qwcT,
                qwcT,
                cst["wcb"]
                .rearrange("p (u v d) -> p u v d", u=1, v=1)
                .to_broadcast([128, BPC, 2, 128]),
            )
            sq_tmp = work.tile([128, BPC, 2, 128], fp32, tag="sq_tmp")
            nc.gpsimd.tensor_mul(
                sq_tmp,
                qry_mega,
                cst["wqb"]
                .rearrange("p (u v d) -> p u v d", u=1, v=1)
                .to_broadcast([128, BPC, 2, 128]),
            )
            sq_col = small.tile([128, BPC, 2], fp32, tag="sq_col")
            nc.vector.tensor_reduce(out=sq_col, in_=sq_tmp, axis=X, op=add)

            cs_sb = [
                small.tile([4, 128], fp32, tag=f"cs{b}", name=f"cs{b}")
                for b in range(BPC)
            ]
            t_g = [
                small.tile([128, 16], fp32, tag=f"t_g{g}", name=f"t_g{g}")
                for g in range(2)
            ]

            def batch_t_cs(b):
                """t columns + n%4 column sums for one batch."""
                ctx_b = out_sb[b][:, :, 0:128]
                cs_ps = ps_cs.tile([4, 4, 128], fp32, tag="cs")
                nc.tensor.matmul(
                    cs_ps, cst["rsel"], ctx_b[:, 0:4, :], start=True, stop=False
                )
                nc.tensor.matmul(
                    cs_ps, cst["rsel"], ctx_b[:, 4:8, :], start=False, stop=True
                )
                nc.vector.tensor_reduce(
                    out=cs_sb[b],
                    in_=cs_ps.rearrange("p k d -> p d k"),
                    axis=X,
                    op=add,
                )
                # t_g[g][p, 8b'+2r+h] = t[r, 128h+p] (s_q added per group)
                ctx_v = ctx_b.rearrange("p (r h) d -> p h r d", h=2)
                t_v = t_g[b // 2][:, 8 * (b % 2) : 8 * (b % 2) + 8].rearrange(
                    "p (r h) -> p h r", h=2
                )
                for h in range(2):
                    g_tmp = work.tile([128, 4, 128], fp32, tag="g_tmp")
                    eng = nc.gpsimd if h == 0 else nc.vector
                    eng.tensor_mul(
                        g_tmp,
                        ctx_v[:, h],
                        qwcT[:, b, h, :]
                        .rearrange("p (u d) -> p u d", u=1)
                        .to_broadcast([128, 4, 128]),
                    )
                    nc.vector.tensor_reduce(
                        out=t_v[:, h], in_=g_tmp, axis=X, op=add
                    )

            # per-group softmax state carried to the batch tails
            scT2 = [None, None]
            sqT2 = [None, None]

            def group_softmax(g):
                """ONE softmax block for batches 2g, 2g+1: row q = 8b'+2r+h."""
                tg = t_g[g]
                nc.gpsimd.tensor_add(
                    tg.rearrange("p (b r h) -> p b r h", b=2, h=2),
                    tg.rearrange("p (b r h) -> p b r h", b=2, h=2),
                    sq_col[:, 2 * g : 2 * g + 2].rearrange(
                        "p b (u h) -> p b u h", u=1
                    ).to_broadcast([128, 2, 4, 2]),
                )
                t16_ps = ps_tr.tile([16, 128], fp32, tag="tr")
                nc.tensor.transpose(t16_ps, tg, cst["ident"])
                e16 = small.tile([16, 128], fp32, tag=f"e16_{g}", name=f"e16_{g}")
                rowsumc = small.tile([16, 1], fp32, tag="rowsumc")
                nc.scalar.activation(
                    out=e16, in_=t16_ps, func=Exp, accum_out=rowsumc
                )
                pairs_ps = ps_sm.tile([8, 1], fp32, tag="sm")
                nc.tensor.matmul(
                    pairs_ps, cst["pairsel"], rowsumc, start=True, stop=True
                )
                rec8 = small.tile([8, 1], fp32, tag="rec8")
                nc.vector.reciprocal(out=rec8, in_=pairs_ps)
                rec16_ps = ps_sm.tile([16, 1], fp32, tag="sm")
                nc.tensor.matmul(
                    rec16_ps, cst["pairselT"], rec8, start=True, stop=True
                )
                rec16 = small.tile([16, 1], fp32, tag="rec16")
                nc.vector.tensor_copy(out=rec16, in_=rec16_ps)
                sc16 = small.tile([16, 128], fp32, tag=f"sc16_{g}", name=f"sc16_{g}")
                nc.scalar.activation(out=sc16, in_=e16, func=Copy, scale=rec16)
                u2_ps = ps_sm.tile([4, 128], fp32, tag="sm")
                nc.tensor.matmul(u2_ps, cst["hsel"], e16, start=True, stop=True)
                u2 = small.tile([4, 128], fp32, tag="u2")
                nc.scalar.copy(out=u2, in_=u2_ps)

                scT_ps = ps_tr.tile([128, 16], fp32, tag="tr")
                nc.tensor.transpose(scT_ps, sc16, cst["i16"])
                scT2[g] = small.tile(
                    [128, 16], fp32, tag=f"scT_{g}", name=f"scT_{g}"
                )
                nc.vector.tensor_copy(out=scT2[g], in_=scT_ps)
                eT_ps = ps_tr.tile([128, 16], fp32, tag="tr")
                nc.tensor.transpose(eT_ps, e16, cst["i16"])
                u2T_ps = ps_tr.tile([128, 4], fp32, tag="tr")
                nc.tensor.transpose(u2T_ps, u2, cst["i16"][:4, :4])
                recu = small.tile([128, 4], fp32, tag="recu")
                nc.vector.reciprocal(out=recu, in_=u2T_ps)
                sqT2[g] = small.tile(
                    [128, 16], fp32, tag=f"sqT_{g}", name=f"sqT_{g}"
                )
                nc.vector.tensor_mul(
                    sqT2[g].rearrange("p (b r h) -> p b r h", b=2, h=2),
                    eT_ps.rearrange("p (b r h) -> p b r h", b=2, h=2),
                    recu.rearrange("p (b u h) -> p b u h", u=1, h=2)
                    .to_broadcast([128, 2, 4, 2]),
                )

            def batch_tail(b):
                g, b2 = b // 2, b % 2
                q0 = 8 * b2
                scT = scT2[g][:, q0 : q0 + 8].rearrange("p (r h) -> p r h", r=4)
                sqT = sqT2[g][:, q0 : q0 + 8].rearrange("p (r h) -> p r h", r=4)

                # SM4T[r',r] = sum_j sq[r',j] sc[r,j], scaled by 1/256
                sm4t_ps = ps_mm.tile([4, 4], fp32, tag="mm")
                for h in range(2):
                    nc.tensor.matmul(
                        sm4t_ps, sqT[:, :, h], scT[:, :, h],
                        start=(h == 0), stop=(h == 1),
                    )
                sm4t = small.tile([4, 4], fp32, tag="sm4t")
                nc.vector.tensor_scalar_mul(sm4t, sm4t_ps, 1.0 / 256.0)

                # C2Q[r,d] = sum_j sc[r,j] qry[j,d]
                c2q_ps = ps_mm.tile([4, 128], fp32, tag="mm")
                for h in range(2):
                    nc.tensor.matmul(
                        c2q_ps, scT[:, :, h], qry_mega[:, b, h, :],
                        start=(h == 0), stop=(h == 1),
                    )
                c2q = small.tile([4, 128], fp32, tag="c2q")
                nc.scalar.copy(out=c2q, in_=c2q_ps)

                # Q2C[r,d] = sum_{r'} SM4[r,r'] CS[r',d]
                q2c_ps = ps_mm.tile([4, 128], fp32, tag="mm")
                nc.tensor.matmul(q2c_ps, sm4t, cs_sb[b], start=True, stop=True)
                q2c = small.tile([4, 128], fp32, tag="q2c")
                nc.scalar.copy(out=q2c, in_=q2c_ps)

                # broadcast rows r -> 128 partitions (p%4 pattern)
                repc_ps = ps_rep.tile([128, 128], fp32, tag="rep")
                nc.tensor.matmul(repc_ps, cst["b4"], c2q, start=True, stop=True)
                repq_ps = ps_rep.tile([128, 128], fp32, tag="rep")
                nc.tensor.matmul(repq_ps, cst["b4"], q2c, start=True, stop=True)
                repq = small.tile([128, 128], fp32, tag="repq")
                nc.scalar.copy(out=repq, in_=repq_ps)

                # assemble remaining output columns and ship the batch
                ctx_b = out_sb[b][:, :, 0:128]
                nc.scalar.copy(
                    out=out_sb[b][:, :, 128:256],
                    in_=repc_ps.rearrange("p (u d) -> p u d", u=1)
                    .to_broadcast([128, 8, 128]),
                )
                nc.vector.tensor_mul(
                    out_sb[b][:, :, 256:384],
                    ctx_b,
                    repc_ps.rearrange("p (u d) -> p u d", u=1)
                    .to_broadcast([128, 8, 128]),
                )
                nc.gpsimd.tensor_mul(
                    out_sb[b][:, :, 384:512],
                    ctx_b,
                    repq.rearrange("p (u d) -> p u d", u=1)
                    .to_broadcast([128, 8, 128]),
                )
                nc.sync.dma_start(
                    out=out_d[b].rearrange("(k p) c -> p k c", p=128),
                    in_=out_sb[b],
                )

            # issue order ~= expected readiness order
            batch_t_cs(0)
            batch_t_cs(1)
            group_softmax(0)
            batch_t_cs(2)
            batch_t_cs(3)
            batch_tail(0)
            batch_tail(1)
            group_softmax(1)
            batch_tail(2)
            batch_tail(3)
    nc.compile()
    return nc


def _get_program():
    global _prog
    if _prog is None:
        _prog = _build_program()
    return _prog


def _make_const_inputs(w):
    w = np.ascontiguousarray(w, dtype=np.float32)
    w_q, w_c, w_m = w[:D, 0], w[D : 2 * D, 0], w[2 * D :, 0]
    p = np.arange(128)
    q = np.arange(16)
    # within a 2-batch group: q = 8b' + 2r + h; pair j = 4b' + r; u k = 2b' + h
    pairsel = (
        (q[:, None] // 8 == np.arange(8)[None, :] // 4)
        & ((q[:, None] % 8) // 2 == np.arange(8)[None, :] % 4)
    ).astype(np.float32)
    hsel = (
        (q[:, None] // 8 == np.arange(4)[None, :] // 2)
        & (q[:, None] % 2 == np.arange(4)[None, :] % 2)
    ).astype(np.float32)
    vals = {
        "ident": np.eye(128, dtype=np.float32),
        "i16": np.eye(16, dtype=np.float32),
        "wmb": np.broadcast_to(w_m[None, :], (128, 128)),
        "wcb": np.broadcast_to(w_c[None, :], (128, 128)),
        "wqb": np.broadcast_to(w_q[None, :], (128, 128)),
        "pairsel": pairsel,
        "pairselT": pairsel.T,
        "hsel": hsel,
        "rsel": (p[:, None] % 4 == np.arange(4)[None, :]).astype(np.float32),
        "b4": (np.arange(4)[:, None] == p[None, :] % 4).astype(np.float32),
    }
    packed = np.zeros((128, _CST_W), dtype=np.float32)
    for n, (parts, c0, cl) in _CST_COLS.items():
        packed[:parts, c0 : c0 + cl] = vals[n]
    return {"cstp": packed}


def _run(context, query, w, trace=False):
    from concourse.bass_utils import run_bass_kernel_spmd

    nc = _get_program()
    context = np.ascontiguousarray(context, dtype=np.float32)
    query = np.ascontiguousarray(query, dtype=np.float32)
    consts = _make_const_inputs(w)

    in_maps = []
    for c in range(NCORES):
        m = {
            "ctx": context[c * BPC : (c + 1) * BPC],
            "qry": query[c * BPC : (c + 1) * BPC],
        }
        m.update(consts)
        in_maps.append(m)

    res = run_bass_kernel_spmd(
        nc, in_maps, core_ids=list(range(NCORES)), trace=trace
    )
    out = np.concatenate([res.results[c]["out"] for c in range(NCORES)], axis=0)
    return out, res


def kernel(context, query, c_mask, q_mask, w):
    out, _ = _run(context, query, w, trace=False)
    return out


# revision 10
# speedup vs baseline: 1.0870x; 1.0333x over previous
"""Trainium2 Bass kernel for nn_ContextQueryAttentionLayer.

Math: with B,N,M,D = 32,1024,256,128 the reference's gather index collapses:
  idx[i,j] = (i*M + j) % N = 256*(i%4) + j          (since M=256, N=1024)
so the similarity matrix S (b,n,m) has only 4 distinct rows per batch,
S[b,i,:] = t[b, i%4, :] with t (4,256):
  t[r,j] = q_j.w_q + c_{256r+j}.w_c + sum_d q_{j,d} w_m_d c_{256r+j,d}
Both softmaxes, c2q, sm (a 4x4 matrix per batch) and q2c then collapse to
rank-4-per-batch quantities, leaving a DMA-bound kernel:
  out[b,n] = [ctx_n, C2Q[n%4], ctx_n*C2Q[n%4], ctx_n*Q2C[n%4]]

Sharding: data-parallel over batch, 4 batches per core on 8 cores.
On-core layout: rows n=128k+p -> partition p (so n%4 == p%4).

v4: output rows assembled in SBUF ([128,8,512]/batch, ctx DMA-loaded
straight into cols 0:128) -> ONE output DMA per batch with 2KB-contiguous
descriptors. DMA triggers are spread across the tensor+sync sequencers
and ctx loads split in halves (descriptor generation ~2.4us/1024 BDs is
serialized per issuing engine). Query prep is per batch so batch 0's
t-columns start as soon as its ctx lands. Softmax runs in two 2-batch
groups; both softmax normalizations are folded into the PSUM->SBUF copy
scales (1/S_r on c2q, 1/(256 S_r) on q2c, 1/U_q into sqT) using the raw
exp matrix, which drops the sc/rec-expansion chain. Column sums via
TensorE rsel-matmul. Products split DVE/GpSimd; issue order tracks
expected readiness (engines execute in order).
"""

import numpy as np

B, N, M, D = 32, 1024, 256, 128
NCORES = 8
BPC = B // NCORES  # batches per core

_prog = None

# packed constant layout: name -> (partitions, col_start, col_len)
_CST_COLS = {
    "ident": (128, 0, 128),
    "wmb": (128, 128, 128),
    "wcb": (128, 256, 128),
    "wqb": (128, 384, 128),
    "b4": (4, 512, 128),
    "i16": (16, 640, 16),
    "pairsel": (16, 656, 8),
    "hsel": (16, 680, 4),
    "rsel": (128, 684, 4),
}
_CST_W = 688


def _build_program():
    import concourse.bacc as bacc
    import concourse.mybir as mybir
    from concourse.tile import TileContext

    fp32 = mybir.dt.float32
    nc = bacc.Bacc("TRN2", target_bir_lowering=False, name="cqattn")

    ctx_d = nc.dram_tensor("ctx", [BPC, N, D], fp32, kind="ExternalInput")
    qry_d = nc.dram_tensor("qry", [BPC, M, D], fp32, kind="ExternalInput")
    cstp_d = nc.dram_tensor("cstp", [128, _CST_W], fp32, kind="ExternalInput")
    out_d = nc.dram_tensor("out", [BPC, N, 4 * D], fp32, kind="ExternalOutput")

    Exp = mybir.ActivationFunctionType.Exp
    Copy = mybir.ActivationFunctionType.Copy
    add = mybir.AluOpType.add
    X = mybir.AxisListType.X

    with TileContext(nc) as tc:
        with (
            tc.tile_pool(name="consts", bufs=1) as consts,
            tc.tile_pool(name="io", bufs=1) as io,
            tc.tile_pool(name="work", bufs=2) as work,
            tc.tile_pool(name="small", bufs=2) as small,
            tc.tile_pool(name="outp", bufs=1) as outp,
            tc.tile_pool(name="ps_tr", bufs=2, space="PSUM") as ps_tr,
            tc.tile_pool(name="ps_sm", bufs=1, space="PSUM") as ps_sm,
            tc.tile_pool(name="ps_mm", bufs=2, space="PSUM") as ps_mm,
            tc.tile_pool(name="ps_cs", bufs=1, space="PSUM") as ps_cs,
            tc.tile_pool(name="ps_rep", bufs=2, space="PSUM") as ps_rep,
        ):
            qry_mega = io.tile([128, BPC, 2, 128], fp32, tag="qry", name="qry_mega")
            out_sb = [
                outp.tile([128, 8, 512], fp32, tag=f"out{b}", name=f"out{b}")
                for b in range(BPC)
            ]

            # ---- loads. Descriptor generation serializes per issuing
            # engine, so split ctx loads in halves and spread triggers:
            # batches 0/1 expand on the (early-idle) Activation sequencer.
            def load_ctx(b, eng):
                for k0 in (0, 4):
                    eng.dma_start(
                        out=out_sb[b][:, k0 : k0 + 4, 0:128],
                        in_=ctx_d[b, 128 * k0 : 128 * (k0 + 4)].rearrange(
                            "(k p) d -> p k d", p=128
                        ),
                    )

            load_ctx(0, nc.scalar)
            load_ctx(1, nc.scalar)
            cstp = consts.tile([128, _CST_W], fp32, tag="cstp", name="cstp")
            nc.sync.dma_start(out=cstp, in_=cstp_d[...])
            cst = {
                n: cstp[:p, c0 : c0 + cl] for n, (p, c0, cl) in _CST_COLS.items()
            }
            for b in range(BPC):
                nc.sync.dma_start(
                    out=qry_mega[:, b],
                    in_=qry_d[b].rearrange("(h p) d -> p h d", p=128),
                )
            load_ctx(2, nc.sync)
            load_ctx(3, nc.sync)

            # ---- per-batch query prep: qwcT_b = qry_b*w_m + w_c (DVE),
            # sq_b = qry_b . w_q (GpS mul + DVE reduce)
            qwcT = work.tile([128, BPC, 2, 128], fp32, tag="qwcT")
            sq_tmp = work.tile([128, BPC, 2, 128], fp32, tag="sq_tmp")
            sq_col = small.tile([128, BPC, 2], fp32, tag="sq_col")
            wmb_b = (
                cst["wmb"].rearrange("p (v d) -> p v d", v=1)
                .to_broadcast([128, 2, 128])
            )
            wcb_b = (
                cst["wcb"].rearrange("p (v d) -> p v d", v=1)
                .to_broadcast([128, 2, 128])
            )
            wqb_b = (
                cst["wqb"].rearrange("p (v d) -> p v d", v=1)
                .to_broadcast([128, 2, 128])
            )

            def prep(b):
                nc.vector.tensor_mul(qwcT[:, b], qry_mega[:, b], wmb_b)
                nc.vector.tensor_add(qwcT[:, b], qwcT[:, b], wcb_b)
                nc.gpsimd.tensor_mul(sq_tmp[:, b], qry_mega[:, b], wqb_b)
                nc.vector.tensor_reduce(
                    out=sq_col[:, b], in_=sq_tmp[:, b], axis=X, op=add
                )

            cs_sb = [
                small.tile([4, 128], fp32, tag=f"cs{b}", name=f"cs{b}")
                for b in range(BPC)
            ]
            t_g = [
                small.tile([128, 16], fp32, tag=f"t_g{g}", name=f"t_g{g}")
                for g in range(2)
            ]

            def batch_t_cs(b):
                """t columns + n%4 column sums for one batch."""
                ctx_b = out_sb[b][:, :, 0:128]
                cs_ps = ps_cs.tile([4, 4, 128], fp32, tag="cs")
                nc.tensor.matmul(
                    cs_ps, cst["rsel"], ctx_b[:, 0:4, :], start=True, stop=False
                )
                nc.tensor.matmul(
                    cs_ps, cst["rsel"], ctx_b[:, 4:8, :], start=False, stop=True
                )
                nc.vector.tensor_reduce(
                    out=cs_sb[b],
                    in_=cs_ps.rearrange("p k d -> p d k"),
                    axis=X,
                    op=add,
                )
                # t_g[g][p, 8b'+2r+h] = t[r, 128h+p] (s_q added per group)
                ctx_v = ctx_b.rearrange("p (r h) d -> p h r d", h=2)
                t_v = t_g[b // 2][:, 8 * (b % 2) : 8 * (b % 2) + 8].rearrange(
                    "p (r h) -> p h r", h=2
                )
                for h in range(2):
                    g_tmp = work.tile([128, 4, 128], fp32, tag="g_tmp")
                    eng = nc.gpsimd if h == 0 else nc.vector
                    eng.tensor_mul(
                        g_tmp,
                        ctx_v[:, h],
                        qwcT[:, b, h, :]
                        .rearrange("p (u d) -> p u d", u=1)
                        .to_broadcast([128, 4, 128]),
                    )
                    nc.vector.tensor_reduce(
                        out=t_v[:, h], in_=g_tmp, axis=X, op=add
                    )

            # per-group softmax state carried to the batch tails
            eT_sb = [None, None]
            sqT2 = [None, None]
            rec4 = [[None, None], [None, None]]   # 1/S_r       per (g, b')
            rec4q = [[None, None], [None, None]]  # 1/(256 S_r) per (g, b')

            def group_softmax(g):
                """Softmax for batches 2g, 2g+1 on raw exp: row q = 8b'+2r+h.

                Normalizations are NOT applied to the exp matrix; 1/S_r and
                1/(256 S_r) ride the c2q/q2c PSUM copies, 1/U_q rides sqT.
                """
                tg = t_g[g]
                nc.gpsimd.tensor_add(
                    tg.rearrange("p (b r h) -> p b r h", b=2, h=2),
                    tg.rearrange("p (b r h) -> p b r h", b=2, h=2),
                    sq_col[:, 2 * g : 2 * g + 2].rearrange(
                        "p b (u h) -> p b u h", u=1
                    ).to_broadcast([128, 2, 4, 2]),
                )
                t16_ps = ps_tr.tile([16, 128], fp32, tag="tr")
                nc.tensor.transpose(t16_ps, tg, cst["ident"])
                e16 = small.tile([16, 128], fp32, tag=f"e16_{g}", name=f"e16_{g}")
                rowsumc = small.tile([16, 1], fp32, tag="rowsumc")
                nc.scalar.activation(
                    out=e16, in_=t16_ps, func=Exp, accum_out=rowsumc
                )
                for b2 in range(2):
                    pairs_ps = ps_sm.tile([4, 1], fp32, tag="sm")
                    nc.tensor.matmul(
                        pairs_ps,
                        cst["pairsel"][:, 4 * b2 : 4 * b2 + 4],
                        rowsumc,
                        start=True,
                        stop=True,
                    )
                    rec4[g][b2] = small.tile(
                        [4, 1], fp32, tag=f"rec4_{g}{b2}", name=f"rec4_{g}{b2}"
                    )
                    nc.vector.reciprocal(out=rec4[g][b2], in_=pairs_ps)
                    rec4q[g][b2] = small.tile(
                        [4, 1], fp32, tag=f"rec4q_{g}{b2}", name=f"rec4q_{g}{b2}"
                    )
                    nc.vector.tensor_scalar_mul(
                        rec4q[g][b2], rec4[g][b2], 1.0 / 256.0
                    )
                u2_ps = ps_sm.tile([4, 128], fp32, tag="sm")
                nc.tensor.matmul(u2_ps, cst["hsel"], e16, start=True, stop=True)
                u2 = small.tile([4, 128], fp32, tag="u2")
                nc.scalar.copy(out=u2, in_=u2_ps)

                eT_ps = ps_tr.tile([128, 16], fp32, tag="tr")
                nc.tensor.transpose(eT_ps, e16, cst["i16"])
                eT_sb[g] = small.tile(
                    [128, 16], fp32, tag=f"eT_{g}", name=f"eT_{g}"
                )
                nc.vector.tensor_copy(out=eT_sb[g], in_=eT_ps)
                u2T_ps = ps_tr.tile([128, 4], fp32, tag="tr")
                nc.tensor.transpose(u2T_ps, u2, cst["i16"][:4, :4])
                recu = small.tile([128, 4], fp32, tag="recu")
                nc.vector.reciprocal(out=recu, in_=u2T_ps)
                sqT2[g] = small.tile(
                    [128, 16], fp32, tag=f"sqT_{g}", name=f"sqT_{g}"
                )
                nc.vector.tensor_mul(
                    sqT2[g].rearrange("p (b r h) -> p b r h", b=2, h=2),
                    eT_ps.rearrange("p (b r h) -> p b r h", b=2, h=2),
                    recu.rearrange("p (b u h) -> p b u h", u=1, h=2)
                    .to_broadcast([128, 2, 4, 2]),
                )

            def batch_tail(b):
                g, b2 = b // 2, b % 2
                q0 = 8 * b2
                eT = eT_sb[g][:, q0 : q0 + 8].rearrange("p (r h) -> p r h", r=4)
                sqT = sqT2[g][:, q0 : q0 + 8].rearrange("p (r h) -> p r h", r=4)

                # raw Gram matrix: sm4t_ps[a,b] = sum_q e[a,q] e[b,q] / U_q
                sm4t_ps = ps_mm.tile([4, 4], fp32, tag="mm")
                for h in range(2):
                    nc.tensor.matmul(
                        sm4t_ps, sqT[:, :, h], eT[:, :, h],
                        start=(h == 0), stop=(h == 1),
                    )
                sm4t = small.tile([4, 4], fp32, tag="sm4t")
                nc.vector.tensor_copy(out=sm4t, in_=sm4t_ps)

                # C2Q[r,d] = (1/S_r) sum_q e[r,q] qry[q,d]
                c2q_ps = ps_mm.tile([4, 128], fp32, tag="mm")
                for h in range(2):
                    nc.tensor.matmul(
                        c2q_ps, eT[:, :, h], qry_mega[:, b, h, :],
                        start=(h == 0), stop=(h == 1),
                    )
                c2q = small.tile([4, 128], fp32, tag="c2q")
                nc.scalar.activation(
                    out=c2q, in_=c2q_ps, func=Copy, scale=rec4[g][b2]
                )

                # Q2C[r,d] = (1/(256 S_r)) sum_{r'} sm4t[r',r] CS[r',d]
                q2c_ps = ps_mm.tile([4, 128], fp32, tag="mm")
                nc.tensor.matmul(q2c_ps, sm4t, cs_sb[b], start=True, stop=True)
                q2c = small.tile([4, 128], fp32, tag="q2c")
                nc.scalar.activation(
                    out=q2c, in_=q2c_ps, func=Copy, scale=rec4q[g][b2]
                )

                # broadcast rows r -> 128 partitions (p%4 pattern)
                repc_ps = ps_rep.tile([128, 128], fp32, tag="rep")
                nc.tensor.matmul(repc_ps, cst["b4"], c2q, start=True, stop=True)
                repq_ps = ps_rep.tile([128, 128], fp32, tag="rep")
                nc.tensor.matmul(repq_ps, cst["b4"], q2c, start=True, stop=True)
                repc_sb = small.tile([128, 128], fp32, tag="repc_sb")
                nc.scalar.copy(out=repc_sb, in_=repc_ps)

                # assemble remaining output columns and ship the batch
                ctx_b = out_sb[b][:, :, 0:128]
                nc.scalar.copy(
                    out=out_sb[b][:, :, 128:256],
                    in_=repc_ps.rearrange("p (u d) -> p u d", u=1)
                    .to_broadcast([128, 8, 128]),
                )
                nc.vector.tensor_mul(
                    out_sb[b][:, 0:4, 256:384],
                    ctx_b[:, 0:4, :],
                    repc_ps.rearrange("p (u d) -> p u d", u=1)
                    .to_broadcast([128, 4, 128]),
                )
                nc.gpsimd.tensor_mul(
                    out_sb[b][:, 4:8, 256:384],
                    ctx_b[:, 4:8, :],
                    repc_sb.rearrange("p (u d) -> p u d", u=1)
                    .to_broadcast([128, 4, 128]),
                )
                nc.vector.tensor_mul(
                    out_sb[b][:, :, 384:512],
                    ctx_b,
                    repq_ps.rearrange("p (u d) -> p u d", u=1)
                    .to_broadcast([128, 8, 128]),
                )
                nc.sync.dma_start(
                    out=out_d[b].rearrange("(k p) c -> p k c", p=128),
                    in_=out_sb[b],
                )

            # issue order ~= expected readiness order
            prep(0)
            prep(1)
            batch_t_cs(0)
            prep(2)
            prep(3)
            batch_t_cs(1)
            group_softmax(0)
            batch_t_cs(2)
            batch_t_cs(3)
            batch_tail(0)
            batch_tail(1)
            group_softmax(1)
            batch_tail(2)
            batch_tail(3)
    nc.compile()
    return nc


def _get_program():
    global _prog
    if _prog is None:
        _prog = _build_program()
    return _prog


def _make_const_inputs(w):
    w = np.ascontiguousarray(w, dtype=np.float32)
    w_q, w_c, w_m = w[:D, 0], w[D : 2 * D, 0], w[2 * D :, 0]
    p = np.arange(128)
    q = np.arange(16)
    # within a 2-batch group: q = 8b' + 2r + h; pair j = 4b' + r; u k = 2b' + h
    pairsel = (
        (q[:, None] // 8 == np.arange(8)[None, :] // 4)
        & ((q[:, None] % 8) // 2 == np.arange(8)[None, :] % 4)
    ).astype(np.float32)
    hsel = (
        (q[:, None] // 8 == np.arange(4)[None, :] // 2)
        & (q[:, None] % 2 == np.arange(4)[None, :] % 2)
    ).astype(np.float32)
    vals = {
        "ident": np.eye(128, dtype=np.float32),
        "i16": np.eye(16, dtype=np.float32),
        "wmb": np.broadcast_to(w_m[None, :], (128, 128)),
        "wcb": np.broadcast_to(w_c[None, :], (128, 128)),
        "wqb": np.broadcast_to(w_q[None, :], (128, 128)),
        "pairsel": pairsel,
        "hsel": hsel,
        "rsel": (p[:, None] % 4 == np.arange(4)[None, :]).astype(np.float32),
        "b4": (np.arange(4)[:, None] == p[None, :] % 4).astype(np.float32),
    }
    packed = np.zeros((128, _CST_W), dtype=np.float32)
    for n, (parts, c0, cl) in _CST_COLS.items():
        packed[:parts, c0 : c0 + cl] = vals[n]
    return {"cstp": packed}


def _run(context, query, w, trace=False):
    from concourse.bass_utils import run_bass_kernel_spmd

    nc = _get_program()
    context = np.ascontiguousarray(context, dtype=np.float32)
    query = np.ascontiguousarray(query, dtype=np.float32)
    consts = _make_const_inputs(w)

    in_maps = []
    for c in range(NCORES):
        m = {
            "ctx": context[c * BPC : (c + 1) * BPC],
            "qry": query[c * BPC : (c + 1) * BPC],
        }
        m.update(consts)
        in_maps.append(m)

    res = run_bass_kernel_spmd(
        nc, in_maps, core_ids=list(range(NCORES)), trace=trace
    )
    out = np.concatenate([res.results[c]["out"] for c in range(NCORES)], axis=0)
    return out, res


def kernel(context, query, c_mask, q_mask, w):
    out, _ = _run(context, query, w, trace=False)
    return out
